# revision 49
# baseline (speedup 1.0000x reference)
"""GAT-style dense attention kernel for TRN2 (8 NeuronCores, SPMD over batch).

Reference computation (B=N=256, F=128, H=4, D=8):
  q = x@Wq+bq; k = x@Wk+bk; v = x@Wv+bv          (per-head dim D=8)
  s = einsum('bqhd,bkhd->bhqk', q, k)/sqrt(D)
  s = where(adj[q,k]==0, -inf, s)                 (adj shared across b,h)
  a = softmax(s, -1)
  out = einsum('bhqk,bkhd->bqhd', a, v).reshape(B,N,H*D) @ Wo + bo

Kernel strategy (per core: 32 batches):
  - host: xT = x.transpose -> [b, F, N] in bf16 so contraction dim F is on
    partitions; all projection weights bf16 (spread layout: head h occupies
    partitions 32h..32h+8; scale 1/sqrt(D) folded into Wq/bq)
  - bias algebra: softmax over k is invariant to per-q-row constants, so the
    k-projection bias drops entirely and the q bias rides the PSUM->SBUF
    copy as a per-partition ACT bias operand (the spurious +bq on the kT
    partitions adds a per-q constant to scores - also softmax-invariant);
    bv@Wo is constant post-normalization and folds into bo
  - scores S^T[k,q] per head: K=8 matmuls, 4 heads in PE row groups
  - exp split across engines, mask folded in per-path:
      pair 0 (heads 0,1): additive -20 mask preloaded in PSUM via one
        identity-matmul per head, then ACT Exp -> bf16
      pair 1 (heads 2,3): DVE scalar_tensor_tensor Schraudolph exp:
        i16 = trunc(S*A16 + msch) bitcast bf16, with the mask and the
        truncation bias folded into the msch plane (no PE mask matmuls)
  - V and Wo fused on host: Wvo_h = Wv_h @ Wo_h; a ones column per head in
    the same stationary operand yields the softmax row-sums
  - P9 matmuls col-packed: head h writes PSUM partitions 32h..32h+9
  - P9 -> bf16 (ACT copy), then DMA xbar transpose back to natural [q, :]
    layout; VectorE: reciprocal of rowsums, scale, sum heads, +bo, DMA out
"""

import sys

sys.path.insert(0, "/opt/trn_rl_repo")

import ml_dtypes
import numpy as np

import concourse.bass as bass
import concourse.tile as tile
from concourse import mybir
from concourse.bass_utils import run_bass_kernel_spmd
from concourse.tile_rust import add_dep_helper


def _dep(from_inst, to_inst, reason):
    if from_inst is None or to_inst is None:
        return
    add_dep_helper(
        getattr(from_inst, "ins", from_inst),
        getattr(to_inst, "ins", to_inst),
        sync=False,
        reason=reason,
    )

B = 256
N = 256
F = 128
H = 4
D = 8
NCORES = 8
BPC = B // NCORES  # batches per core
MASK_NEG = -20.0
# Schraudolph bf16-exp constants: i16 = trunc(s * A16 + B16) bitcast bf16.
# B16 includes +0.5 so that truncation acts as round-to-nearest.
A16 = 184.6618
B16 = 16250.5

f32 = mybir.dt.float32
f32r = mybir.dt.float32r
bf16 = mybir.dt.bfloat16
i16 = mybir.dt.int16

# cblob f32-column layout
C_MSCH = 0           # [128, 512] f32 schraudolph plane (c, q), pair-1 path
C_ADJ = 512          # [128, 512] f32r additive mask (c, q) MASK_NEG/0
C_IDENT = 1024       # [128, 128] f32r identity (mask matmul stationary)
C_WQS = 1152         # [128, 128] f32r
C_WKS = 1280         # [128, 128] f32r
C_WVO = 1408         # [128, 256] f32r (cols 128:256 zero pad)
C_BO = 1664          # [128, 8] f32 (bo + sum_h bv_h@Wo_h, bcast to partitions)
C_BQ = 1672          # [128, 1] f32 spread q bias (ACT bias operand)
C_TOT = 1673
# crow f32-column layout
R_ONES = 0           # [1, 128] bf16 -> 64 f32 cols
R_BVO = 64           # [1, 128] bf16 -> 64 f32 cols (ones columns only)
R_TOT = 128


def _pack_bf16(a):
    """Pack a [r, 2c] bf16 array into [r, c] f32 (low half = even cols)."""
    b = np.ascontiguousarray(a.astype(ml_dtypes.bfloat16)).view(np.uint16)
    packed = b[:, 0::2].astype(np.uint32) | (b[:, 1::2].astype(np.uint32) << 16)
    return packed.view(np.float32)


def _build_consts(edge_index, Wq, bq, Wk, bk, Wv, bv, Wo, bo):
    scale = 1.0 / np.sqrt(np.float32(D))

    # spread projection weights: output partition 32h+d holds head h, dim d
    Wq_s = np.zeros((F, 128), np.float32)
    Wk_s = np.zeros((F, 128), np.float32)
    bq_s = np.zeros((128, 1), np.float32)
    for h in range(H):
        for d in range(D):
            Wq_s[:, 32 * h + d] = Wq[:, 8 * h + d] * scale
            Wk_s[:, 32 * h + d] = Wk[:, 8 * h + d]
            bq_s[32 * h + d, 0] = bq[8 * h + d] * scale

    # fused V*Wo, head h occupies cols 32h..32h+31: col 32h gets the ones
    # (from the crow ones-matmul), cols 32h+1..8 the fused V@Wo, the rest
    # zero padding (so P9 writes whole 32-partition groups). bv@Wo folds
    # into the output bias.
    Wvo = np.zeros((F, 128), np.float32)
    bo_eff = bo.astype(np.float32).copy()
    for h in range(H):
        wv_h = Wv[:, 8 * h : 8 * h + 8]  # [F, 8]
        wo_h = Wo[8 * h : 8 * h + 8, :]  # [8, 8]
        Wvo[:, 32 * h + 1 : 32 * h + 9] = wv_h @ wo_h
        bo_eff += bv[8 * h : 8 * h + 8] @ wo_h
    bvo_ones = np.zeros((1, 128), np.float32)
    for h in range(H):
        bvo_ones[0, 32 * h] = 1.0

    # adjacency; mask addend M^T[k, q] packed as [128, (c, q)]
    adj = np.zeros((B, B), np.float32)
    adj[edge_index[0], edge_index[1]] = 1.0
    adjT_p = np.ascontiguousarray(
        adj.T.reshape(2, 128, 256).transpose(1, 0, 2)
    ).reshape(128, 512)
    maskT_p = np.where(adjT_p != 0.0, np.float32(0.0), np.float32(MASK_NEG))
    msch = np.where(
        adj.T == 0.0, np.float32(B16 + A16 * MASK_NEG), np.float32(B16)
    )
    msch_p = np.ascontiguousarray(
        msch.reshape(2, 128, 256).transpose(1, 0, 2)
    ).reshape(128, 512)

    bo_b = np.broadcast_to(bo_eff, (128, D)).copy()

    cblob = np.zeros((128, C_TOT), np.float32)
    cblob[:, C_MSCH : C_MSCH + 512] = msch_p
    cblob[:, C_ADJ : C_ADJ + 512] = maskT_p
    cblob[:, C_IDENT : C_IDENT + 128] = np.eye(128, dtype=np.float32)
    cblob[:, C_WQS : C_WQS + 128] = Wq_s
    cblob[:, C_WKS : C_WKS + 128] = Wk_s
    cblob[:, C_WVO : C_WVO + 128] = Wvo
    cblob[:, C_BO : C_BO + 8] = bo_b
    cblob[:, C_BQ : C_BQ + 1] = bq_s

    crow = np.zeros((1, R_TOT), np.float32)
    crow[:, R_ONES : R_ONES + 64] = _pack_bf16(np.ones((1, 128), np.float32))
    crow[:, R_BVO : R_BVO + 64] = _pack_bf16(bvo_ones)
    return dict(cblob=np.ascontiguousarray(cblob), crow=np.ascontiguousarray(crow))


def _split_excess_waits(nc, max_waits=1):
    """Walrus allows only 2 sync-wait slots per engine instruction. Tile's
    vector-clock wait emission occasionally exceeds that (schedule-dependent);
    hoist the excess onto injected same-engine NoOps placed just before."""
    f = nc.m.functions[0]
    for bb in f.blocks:
        insts = list(bb.instructions)
        n_inserted = 0
        for idx, inst in enumerate(insts):
            si = getattr(inst, "sync_info", None)
            if si is None or not si.on_wait or len(si.on_wait) <= max_waits:
                continue
            waits = list(si.on_wait)
            keep, excess = waits[:max_waits], waits[max_waits:]
            pos = idx + n_inserted
            while excess:
                chunk, excess = excess[:max_waits], excess[max_waits:]
                nop = mybir.InstNoOp(
                    name=nc.get_next_instruction_name(),
                    ins=[],
                    outs=[],
                    engine=inst.engine,
                    sync_info=mybir.SyncInfo(on_wait=chunk, on_update=[]),
                    bass_nofuse=True,
                )
                bb.instructions.insert(pos, nop)
                pos += 1
                n_inserted += 1
            inst.sync_info = mybir.SyncInfo(on_wait=keep, on_update=si.on_update)


def _build_program():
    nc = bass.Bass()

    x_t = nc.declare_dram_parameter("xt", [BPC, F, N], f32r, isOutput=False)
    out = nc.declare_dram_parameter("out", [BPC, N, D], f32, isOutput=True)
    c_blob = nc.declare_dram_parameter("cblob", [128, C_TOT], f32r, isOutput=False)
    c_row = nc.declare_dram_parameter("crow", [1, R_TOT], f32r, isOutput=False)

    with tile.TileContext(nc) as tc:
        with (
            tc.tile_pool(name="consts", bufs=1) as cpool,
            tc.tile_pool(name="xt", bufs=3) as xt_pool,
            tc.tile_pool(name="qk", bufs=3) as qk_pool,
            tc.tile_pool(name="vw", bufs=3) as vw_pool,
            tc.tile_pool(name="E", bufs=3) as e_pool,
            tc.tile_pool(name="p9", bufs=2) as p9_pool,
            tc.tile_pool(name="pnat", bufs=2) as pnat_pool,
            tc.tile_pool(name="small", bufs=4) as sm_pool,
            tc.tile_pool(name="ostage", bufs=3) as ost_pool,
            tc.tile_pool(name="ps_qk", bufs=2, space="PSUM") as ps_qk,
            tc.tile_pool(name="ps_vw", bufs=1, space="PSUM") as ps_vw,
            tc.tile_pool(name="ps_s", bufs=2, space="PSUM") as ps_s,
            tc.tile_pool(name="ps_p9", bufs=1, space="PSUM") as ps_p9,
        ):
            cblob = cpool.tile([128, C_TOT], f32r, tag="cblob")
            nc.sync.dma_start(out=cblob[:], in_=c_blob[:])
            crow = cpool.tile([1, R_TOT], f32r, tag="crow")
            nc.sync.dma_start(out=crow[:], in_=c_row[:])

            msch = cblob[:, C_MSCH : C_MSCH + 512].bitcast(f32)
            maskf = cblob[:, C_ADJ : C_ADJ + 512]                 # [128, 512]
            identr = cblob[:, C_IDENT : C_IDENT + 128]
            wqs = cblob[:, C_WQS : C_WQS + 128]                   # [128, 128]
            wks = cblob[:, C_WKS : C_WKS + 128]                   # [128, 128]
            wvo = cblob[:, C_WVO : C_WVO + 256]                   # [128, 256]
            bob = cblob[:, C_BO : C_BO + 8].bitcast(f32)          # [128, 8]
            bqv = cblob[:, C_BQ : C_BQ + 1].bitcast(f32)          # [128, 1]
            ones = crow[:, R_ONES : R_ONES + 64].bitcast(bf16)    # [1, 128]
            bvo = crow[:, R_BVO : R_BVO + 64].bitcast(bf16)       # [1, 128]

            # Make DVE/ACT/Pool observe the const-DMA queues once, so the
            # const-load ticks drop out of every later wait list.
            obs = cpool.tile([1, 8], f32, tag="obs")
            nc.vector.tensor_copy(obs[:, 0:2], cblob[0:1, 0:2].bitcast(f32))
            nc.vector.tensor_copy(obs[:, 2:4], crow[:, 0:2].bitcast(f32))
            nc.scalar.copy(obs[:, 4:6], cblob[0:1, 2:4].bitcast(f32))
            nc.scalar.copy(obs[:, 6:8], crow[:, 2:4].bitcast(f32))

            msch2 = msch.rearrange("p (c q) -> p c q", c=2)
            mask2 = maskf.rearrange("p (c q) -> p c q", c=2)

            xt_tiles = {}

            def load_xt_group(g):
                t = xt_pool.tile([128, 4, 2, 128], f32r, tag="xt")
                nc.sync.dma_start(
                    out=t[:],
                    in_=x_t[4 * g : 4 * (g + 1)].rearrange(
                        "b f (c n) -> f b c n", c=2
                    ),
                )
                xt_tiles[g] = t

            def stage_compute(b):
                """Projections, copies, scores, exp for batch b (xt already
                loaded). P9(b-1) matmuls are emitted by the caller between
                proj and scores to keep PE dense."""
                xt_sb = xt_tiles[b // 4][:, b % 4]      # [128, 2, 128]
                xt_flat = xt_sb.rearrange("f c n -> f (c n)")

                ps_q = ps_qk.tile([128, 512], f32, tag="qk")
                nc.tensor.matmul(ps_q[:, 0:256], wqs, xt_flat,
                                 start=True, stop=False)
                i_projk = nc.tensor.matmul(ps_q[:, 256:512], wks, xt_flat,
                                           start=False, stop=True)
                # PE-order hint: run this batch's projections before the
                # previous batch's pair-0 P9 so the ACT qk-copy isn't starved
                _dep(p9_stage.get("p0_first"), i_projk, "proj before P9 p0")
                ps_v = ps_vw.tile([128, 512], f32, tag="vw")
                for c in range(2):
                    nc.tensor.matmul(
                        ps_v[:, 256 * c : 256 * (c + 1)], xt_sb[:, c, :], wvo,
                        start=(c == 0), stop=(c == 1),
                    )

                # qT/kT -> SBUF bf16 with the spread q bias applied per
                # partition (the +bq on kT rows adds a per-q-row constant to
                # scores, which softmax cancels).
                qk_sb = qk_pool.tile([128, 512], f32r, tag="qk")
                nc.scalar.activation(
                    qk_sb[:], ps_q[:, 0:512],
                    mybir.ActivationFunctionType.Identity, bias=bqv,
                )
                return {"qk": qk_sb, "ps_v": ps_v}

            def stage_vwcopy(st):
                vw_sb = vw_pool.tile([128, 2, 128], bf16, tag="vw")
                nc.vector.tensor_copy(
                    vw_sb[:],
                    st["ps_v"][:].rearrange("p (c v) -> p c v", c=2)[:, :, 0:128],
                )
                # softmax row-sum ones columns (col 32h of each head group)
                nc.vector.memset(
                    vw_sb.rearrange("p c (h r) -> p c h r", r=32)[:, :, :, 0:1],
                    1.0,
                )
                st["vw"] = vw_sb

            def stage_scores(st, border):
                """Scores + exp for both pairs. The pair -> PSUM-slot
                assignment alternates per batch so the slot-release cycle
                interleaves the ACT-exp and DVE-STT drains (emission order
                stays fixed)."""
                e_parts = {}
                tiles = {}
                for p in border:
                    t = ps_s.tile([128, 2, 2, 256], f32, tag="S")
                    tiles[p] = t
                for p in range(2):
                    qk_sb = st["qk"]
                    ps_sp = tiles[p]
                    for hh in range(2):
                        h = 2 * p + hh
                        for c in range(2):
                            nc.tensor.matmul(
                                ps_sp[:, hh, c, :],
                                qk_sb[32 * h : 32 * h + 8,
                                      256 + 128 * c : 384 + 128 * c],
                                qk_sb[32 * h : 32 * h + 8, 0:256],
                                start=(c == 0),
                                stop=(p == 1 and c == 1),
                                skip_group_check=(p == 0 and c == 1),
                                tile_position=(32 * h, 0),
                            )
                    if p == 0:
                        # additive MASK_NEG plane accumulated after the
                        # scores (identity matmul); closes each bank's group
                        for hh in range(2):
                            for c in range(2):
                                nc.tensor.matmul(
                                    ps_sp[:, hh, c, :],
                                    identr, mask2[:, c],
                                    start=False, stop=(c == 1),
                                    skip_group_check=(c == 0),
                                )
                        e_raw = e_pool.tile([128, 2, 2, 256], bf16, tag="E0")
                        nc.scalar.activation(
                            e_raw[:], ps_sp[:], mybir.ActivationFunctionType.Exp
                        )
                        e_parts[0] = e_raw
                    else:
                        # DVE Schraudolph bf16-exp with mask folded into msch
                        e_i = e_pool.tile([128, 2, 2, 256], i16, tag="E1")
                        for hh in range(2):
                            i_stt = nc.vector.scalar_tensor_tensor(
                                e_i[:, hh], ps_sp[:, hh], float(A16),
                                msch2,
                                mybir.AluOpType.mult, mybir.AluOpType.add,
                            )
                            st["stt_last"] = i_stt
                        e_parts[1] = e_i.bitcast(bf16)
                st["e"] = e_parts

            p9_stage = {}

            def stage_p9_mm(st, ps_mix, p):
                e_p = st["e"][p]
                first = None
                for hh in range(2):
                    h = 2 * p + hh
                    for c in range(2):
                        i_mm = nc.tensor.matmul(
                            ps_mix[32 * h : 32 * h + 32, 0:256],
                            st["vw"][:, c, 32 * h : 32 * h + 32],
                            e_p[:, hh, c, :],
                            start=(c == 0), stop=(c == 1),
                            tile_position=(0, 32 * h),
                        )
                        first = first or i_mm
                if p == 0:
                    p9_stage["p0_first"] = first

            def stage_p9_fin(j):
                """bf16 copy into the 4-batch staging tile; one DMA xbar
                transpose per 4 batches."""
                ps_mix = p9_stage["mix"]
                if j % 4 == 0:
                    p9s = p9_pool.tile([128, 4, 256], bf16, tag="p9")
                    p9_stage["t"] = p9s
                nc.scalar.copy(p9_stage["t"][:, j % 4, :], ps_mix[:, 0:256])
                if j % 4 == 3:
                    pn4 = pnat_pool.tile([128, 8, 128], bf16, tag="pn")
                    # issue from the ACT hwdge queue: same in-order queue as
                    # the staging copies, so the read can't race them
                    nc.scalar.dma_start_transpose(
                        out=pn4[:],
                        in_=p9_stage["t"].rearrange("p b q -> p (b q)"),
                    )
                    return pn4
                return None

            ostage = {}

            def stage_norm(pn4, g, after=None):
                """Normalize + head-sum + bias for batch group g (4 batches);
                DMA out every 2 groups."""
                pn_r = pn4.rearrange("p bc (h r) -> p bc h r", r=32)
                rec = sm_pool.tile([128, 8, H], f32, tag="rec")
                i_rec = nc.vector.reciprocal(rec[:], pn_r[:, :, :, 0])
                _dep(i_rec, after, "keep norm behind this batch's STT on DVE")
                tmp = sm_pool.tile([128, 8, D, H], f32, tag="tmp")
                nc.vector.tensor_mul(
                    tmp[:],
                    pn_r[:, :, :, 1:9].transpose([0, 1, 3, 2]),
                    rec[:].unsqueeze(2).to_broadcast([128, 8, D, H]),
                )
                red = sm_pool.tile([128, 8, D], f32, tag="red")
                nc.vector.tensor_reduce(
                    red[:], tmp[:], axis=mybir.AxisListType.X,
                    op=mybir.AluOpType.add,
                )
                if g % 2 == 0:
                    ost = ost_pool.tile([128, 8, 2, D], f32, tag="ost")
                    ostage["t"] = ost
                o = 4 * (g % 2)
                nc.vector.tensor_add(
                    ostage["t"][:, o : o + 4, :, :],
                    red[:].rearrange("p (b c) d -> p b c d", c=2),
                    bob.unsqueeze(1).unsqueeze(1).to_broadcast([128, 4, 2, D]),
                )
                if g % 2 == 1:
                    j0 = 8 * (g // 2)
                    nc.sync.dma_start(
                        out=out[j0 : j0 + 8].rearrange(
                            "b (c p) j -> p b c j", c=2
                        ),
                        in_=ostage["t"][:],
                    )

            # software pipeline: iteration b emits
            #   xt prefetch | proj(b) | P9(b-1) | scores(b) | norm(group)
            load_xt_group(0)
            prev_st = None
            pend_pn = []   # [(pn4, group)] awaiting normalize
            for b in range(BPC):
                if (b + 2) % 4 == 0 and (b + 2) // 4 < BPC // 4:
                    load_xt_group((b + 2) // 4)
                st = stage_compute(b)
                if prev_st is not None:
                    mix = ps_p9.tile([128, 512], f32, tag="p9")
                    p9_stage["mix"] = mix
                    stage_p9_mm(prev_st, mix, 1)
                stage_scores(st, (0, 1) if b % 2 == 0 else (1, 0))
                stage_vwcopy(st)
                if prev_st is not None:
                    stage_p9_mm(prev_st, p9_stage["mix"], 0)
                    pn4 = stage_p9_fin(b - 1)
                    if pn4 is not None:
                        pend_pn.append((pn4, (b - 1) // 4))
                # normalize with slack behind the transpose to cover DMA
                # latency
                if len(pend_pn) >= 2 or (pend_pn and b % 4 == 1):
                    pn4, g = pend_pn.pop(0)
                    stage_norm(pn4, g, st.get("stt_last"))
                prev_st = st
            mix = ps_p9.tile([128, 512], f32, tag="p9")
            p9_stage["mix"] = mix
            stage_p9_mm(prev_st, mix, 1)
            stage_p9_mm(prev_st, mix, 0)
            pn4 = stage_p9_fin(BPC - 1)
            if pn4 is not None:
                pend_pn.append((pn4, (BPC - 1) // 4))
            for pn4, g in pend_pn:
                stage_norm(pn4, g)

    _split_excess_waits(nc)
    return nc


_NC_CACHE = None
LAST_RESULTS = None


def kernel(**inputs) -> np.ndarray:
    global _NC_CACHE
    x = np.asarray(inputs["x"], np.float32)
    edge_index = np.asarray(inputs["edge_index"])
    consts = _build_consts(
        edge_index,
        np.asarray(inputs["Wq"], np.float32), np.asarray(inputs["bq"], np.float32),
        np.asarray(inputs["Wk"], np.float32), np.asarray(inputs["bk"], np.float32),
        np.asarray(inputs["Wv"], np.float32), np.asarray(inputs["bv"], np.float32),
        np.asarray(inputs["Wo"], np.float32), np.asarray(inputs["bo"], np.float32),
    )

    if _NC_CACHE is None:
        _NC_CACHE = _build_program()
    nc = _NC_CACHE

    in_maps = []
    for core in range(NCORES):
        xs = x[core * BPC : (core + 1) * BPC]  # [BPC, N, F]
        xt = np.ascontiguousarray(xs.transpose(0, 2, 1)).astype(np.float32)
        m = {"xt": xt}
        m.update(consts)
        in_maps.append(m)

    res = run_bass_kernel_spmd(nc, in_maps, list(range(NCORES)))
    global LAST_RESULTS
    LAST_RESULTS = res
    outs = [res.results[i]["out"] for i in range(NCORES)]
    return np.concatenate(outs, axis=0).astype(np.float32)


if __name__ == "__main__":
    rng = np.random.default_rng(0)
    demo = dict(
        x=rng.standard_normal((B, N, F), dtype=np.float32),
        edge_index=np.concatenate(
            [rng.integers(0, B, (2, 8192)), np.stack([np.arange(B)] * 2)], axis=1
        ).astype(np.int32),
        Wq=rng.standard_normal((F, H * D), dtype=np.float32) / np.sqrt(F),
        bq=rng.standard_normal(H * D, dtype=np.float32) / np.sqrt(F),
        Wk=rng.standard_normal((F, H * D), dtype=np.float32) / np.sqrt(F),
        bk=rng.standard_normal(H * D, dtype=np.float32) / np.sqrt(F),
        Wv=rng.standard_normal((F, H * D), dtype=np.float32) / np.sqrt(F),
        bv=rng.standard_normal(H * D, dtype=np.float32) / np.sqrt(F),
        Wo=rng.standard_normal((H * D, D), dtype=np.float32) / np.sqrt(H * D),
        bo=rng.standard_normal(D, dtype=np.float32) / np.sqrt(H * D),
    )
    out = kernel(**demo)
    print("kernel output", out.shape, out.dtype)


# revision 53
# speedup vs baseline: 1.0847x; 1.0847x over previous
"""GAT-style dense attention kernel for TRN2 (8 NeuronCores, SPMD over batch).

Reference computation (B=N=256, F=128, H=4, D=8):
  q = x@Wq+bq; k = x@Wk+bk; v = x@Wv+bv          (per-head dim D=8)
  s = einsum('bqhd,bkhd->bhqk', q, k)/sqrt(D)
  s = where(adj[q,k]==0, -inf, s)                 (adj shared across b,h)
  a = softmax(s, -1)
  out = einsum('bhqk,bkhd->bqhd', a, v).reshape(B,N,H*D) @ Wo + bo

Kernel strategy (per core: 32 batches):
  - host: xT = x.transpose -> [b, F, N] in bf16 so contraction dim F is on
    partitions; all projection weights bf16 (spread layout: head h occupies
    partitions 32h..32h+8; scale 1/sqrt(D) folded into Wq/bq)
  - bias algebra: softmax over k is invariant to per-q-row constants, so the
    k-projection bias drops entirely and the q bias rides the PSUM->SBUF
    copy as a per-partition ACT bias operand (the spurious +bq on the kT
    partitions adds a per-q constant to scores - also softmax-invariant);
    bv@Wo is constant post-normalization and folds into bo
  - scores S^T[k,q] per head: K=8 matmuls, 4 heads in PE row groups
  - exp split across engines, mask folded in per-path:
      pair 0 (heads 0,1): additive -20 mask preloaded in PSUM via one
        identity-matmul per head, then ACT Exp -> bf16
      pair 1 (heads 2,3): DVE scalar_tensor_tensor Schraudolph exp:
        i16 = trunc(S*A16 + msch) bitcast bf16, with the mask and the
        truncation bias folded into the msch plane (no PE mask matmuls)
  - V and Wo fused on host: Wvo_h = Wv_h @ Wo_h; a ones column per head in
    the same stationary operand yields the softmax row-sums
  - P9 matmuls col-packed: head h writes PSUM partitions 32h..32h+9
  - P9 -> bf16 (ACT copy), then DMA xbar transpose back to natural [q, :]
    layout; VectorE: reciprocal of rowsums, scale, sum heads, +bo, DMA out
"""

import sys

sys.path.insert(0, "/opt/trn_rl_repo")

import ml_dtypes
import numpy as np

import concourse.bass as bass
import concourse.tile as tile
from concourse import mybir
from concourse.bass_utils import run_bass_kernel_spmd
from concourse.tile_rust import add_dep_helper


def _dep(from_inst, to_inst, reason):
    if from_inst is None or to_inst is None:
        return
    add_dep_helper(
        getattr(from_inst, "ins", from_inst),
        getattr(to_inst, "ins", to_inst),
        sync=False,
        reason=reason,
    )

B = 256
N = 256
F = 128
H = 4
D = 8
NCORES = 8
BPC = B // NCORES  # batches per core
MASK_NEG = -20.0
# Schraudolph bf16-exp constants: i16 = trunc(s * A16 + B16) bitcast bf16.
# B16 includes +0.5 so that truncation acts as round-to-nearest.
A16 = 184.6618
B16 = 16250.5

f32 = mybir.dt.float32
f32r = mybir.dt.float32r
bf16 = mybir.dt.bfloat16
i16 = mybir.dt.int16

# cblob f32-column layout
C_MSCH = 0           # [128, 512] f32 schraudolph plane (c, q), pair-1 path
C_ADJ = 512          # [128, 512] f32r additive mask (c, q) MASK_NEG/0
C_IDENT = 1024       # [128, 128] f32r identity (mask matmul stationary)
C_WQS = 1152         # [128, 128] f32r
C_WKS = 1280         # [128, 128] f32r
C_WVO = 1408         # [128, 256] f32r (cols 128:256 zero pad)
C_BO = 1664          # [128, 8] f32 (bo + sum_h bv_h@Wo_h, bcast to partitions)
C_BQ = 1672          # [128, 1] f32 spread q bias (ACT bias operand)
C_IDR = 1673         # [64p, 2, 128] fp8e4 interleaved identity -> 64 f32 cols
C_MDR = 1737         # [64p, 2, 2, 256] fp8e4 interleaved mask -> 256 f32 cols
C_TOT = 1993
# crow f32-column layout
R_ONES = 0           # [1, 128] bf16 -> 64 f32 cols
R_BVO = 64           # [1, 128] bf16 -> 64 f32 cols (ones columns only)
R_TOT = 128


def _pack_fp8(a):
    """Pack a [r, 4c] array into [r, c] f32 via float8_e4m3fn bytes."""
    b = np.ascontiguousarray(a.astype(ml_dtypes.float8_e4m3fn)).view(np.uint8)
    packed = (b[:, 0::4].astype(np.uint32)
              | (b[:, 1::4].astype(np.uint32) << 8)
              | (b[:, 2::4].astype(np.uint32) << 16)
              | (b[:, 3::4].astype(np.uint32) << 24))
    return packed.view(np.float32)


def _pack_bf16(a):
    """Pack a [r, 2c] bf16 array into [r, c] f32 (low half = even cols)."""
    b = np.ascontiguousarray(a.astype(ml_dtypes.bfloat16)).view(np.uint16)
    packed = b[:, 0::2].astype(np.uint32) | (b[:, 1::2].astype(np.uint32) << 16)
    return packed.view(np.float32)


def _build_consts(edge_index, Wq, bq, Wk, bk, Wv, bv, Wo, bo):
    scale = 1.0 / np.sqrt(np.float32(D))

    # spread projection weights: output partition 32h+d holds head h, dim d
    Wq_s = np.zeros((F, 128), np.float32)
    Wk_s = np.zeros((F, 128), np.float32)
    bq_s = np.zeros((128, 1), np.float32)
    for h in range(H):
        for d in range(D):
            Wq_s[:, 32 * h + d] = Wq[:, 8 * h + d] * scale
            Wk_s[:, 32 * h + d] = Wk[:, 8 * h + d]
            bq_s[32 * h + d, 0] = bq[8 * h + d] * scale

    # fused V*Wo, head h occupies cols 32h..32h+31: col 32h gets the ones
    # (from the crow ones-matmul), cols 32h+1..8 the fused V@Wo, the rest
    # zero padding (so P9 writes whole 32-partition groups). bv@Wo folds
    # into the output bias.
    Wvo = np.zeros((F, 128), np.float32)
    bo_eff = bo.astype(np.float32).copy()
    for h in range(H):
        wv_h = Wv[:, 8 * h : 8 * h + 8]  # [F, 8]
        wo_h = Wo[8 * h : 8 * h + 8, :]  # [8, 8]
        Wvo[:, 32 * h + 1 : 32 * h + 9] = wv_h @ wo_h
        bo_eff += bv[8 * h : 8 * h + 8] @ wo_h
    bvo_ones = np.zeros((1, 128), np.float32)
    for h in range(H):
        bvo_ones[0, 32 * h] = 1.0

    # adjacency; mask addend M^T[k, q] packed as [128, (c, q)]
    adj = np.zeros((B, B), np.float32)
    adj[edge_index[0], edge_index[1]] = 1.0
    adjT_p = np.ascontiguousarray(
        adj.T.reshape(2, 128, 256).transpose(1, 0, 2)
    ).reshape(128, 512)
    maskT_p = np.where(adjT_p != 0.0, np.float32(0.0), np.float32(MASK_NEG))
    msch = np.where(
        adj.T == 0.0, np.float32(B16 + A16 * MASK_NEG), np.float32(B16)
    )
    msch_p = np.ascontiguousarray(
        msch.reshape(2, 128, 256).transpose(1, 0, 2)
    ).reshape(128, 512)

    bo_b = np.broadcast_to(bo_eff, (128, D)).copy()

    cblob = np.zeros((128, C_TOT), np.float32)
    cblob[:, C_MSCH : C_MSCH + 512] = msch_p
    cblob[:, C_ADJ : C_ADJ + 512] = maskT_p
    cblob[:, C_IDENT : C_IDENT + 128] = np.eye(128, dtype=np.float32)
    # fp8e4 DoubleRow operands: partition k holds rows 2k/2k+1 interleaved
    cblob[0:64, C_IDR : C_IDR + 64] = _pack_fp8(
        np.eye(128, dtype=np.float32).reshape(64, 256)
    )
    mask_dr = np.stack(
        [maskT_p[:, 256 * c : 256 * (c + 1)].reshape(64, 512) for c in range(2)],
        axis=1,
    ).reshape(64, 1024)
    cblob[0:64, C_MDR : C_MDR + 256] = _pack_fp8(mask_dr)
    cblob[:, C_WQS : C_WQS + 128] = Wq_s
    cblob[:, C_WKS : C_WKS + 128] = Wk_s
    cblob[:, C_WVO : C_WVO + 128] = Wvo
    cblob[:, C_BO : C_BO + 8] = bo_b
    cblob[:, C_BQ : C_BQ + 1] = bq_s

    ident_drn = np.eye(128, dtype=np.float32).reshape(64, 2, 128)
    mask_drn = np.stack(
        [maskT_p[:, 256 * c : 256 * (c + 1)].reshape(64, 2, 256) for c in range(2)],
        axis=1,
    )  # [64, 2(c), 2(i), 256]

    crow = np.zeros((1, R_TOT), np.float32)
    crow[:, R_ONES : R_ONES + 64] = _pack_bf16(np.ones((1, 128), np.float32))
    crow[:, R_BVO : R_BVO + 64] = _pack_bf16(bvo_ones)
    return dict(
        cblob=np.ascontiguousarray(cblob), crow=np.ascontiguousarray(crow),
        cdrl=np.ascontiguousarray(ident_drn).astype(ml_dtypes.float8_e4m3fn),
        cdrm=np.ascontiguousarray(mask_drn).astype(ml_dtypes.float8_e4m3fn),
    )


def _split_excess_waits(nc, max_waits=1):
    """Walrus allows only 2 sync-wait slots per engine instruction. Tile's
    vector-clock wait emission occasionally exceeds that (schedule-dependent);
    hoist the excess onto injected same-engine NoOps placed just before."""
    f = nc.m.functions[0]
    for bb in f.blocks:
        insts = list(bb.instructions)
        n_inserted = 0
        for idx, inst in enumerate(insts):
            si = getattr(inst, "sync_info", None)
            if si is None or not si.on_wait or len(si.on_wait) <= max_waits:
                continue
            waits = list(si.on_wait)
            keep, excess = waits[:max_waits], waits[max_waits:]
            pos = idx + n_inserted
            while excess:
                chunk, excess = excess[:max_waits], excess[max_waits:]
                nop = mybir.InstNoOp(
                    name=nc.get_next_instruction_name(),
                    ins=[],
                    outs=[],
                    engine=inst.engine,
                    sync_info=mybir.SyncInfo(on_wait=chunk, on_update=[]),
                    bass_nofuse=True,
                )
                bb.instructions.insert(pos, nop)
                pos += 1
                n_inserted += 1
            inst.sync_info = mybir.SyncInfo(on_wait=keep, on_update=si.on_update)


def _build_program():
    nc = bass.Bass()

    x_t = nc.declare_dram_parameter("xt", [BPC, F, N], f32r, isOutput=False)
    out = nc.declare_dram_parameter("out", [BPC, N, D], f32, isOutput=True)
    c_blob = nc.declare_dram_parameter("cblob", [128, C_TOT], f32r, isOutput=False)
    c_row = nc.declare_dram_parameter("crow", [1, R_TOT], f32r, isOutput=False)
    fp8e4 = mybir.dt.float8e4
    c_drl = nc.declare_dram_parameter("cdrl", [64, 2, 128], fp8e4, isOutput=False)
    c_drm = nc.declare_dram_parameter("cdrm", [64, 2, 2, 256], fp8e4, isOutput=False)

    with tile.TileContext(nc) as tc:
        with (
            tc.tile_pool(name="consts", bufs=1) as cpool,
            tc.tile_pool(name="xt", bufs=3) as xt_pool,
            tc.tile_pool(name="qk", bufs=3) as qk_pool,
            tc.tile_pool(name="vw", bufs=3) as vw_pool,
            tc.tile_pool(name="E", bufs=3) as e_pool,
            tc.tile_pool(name="p9", bufs=2) as p9_pool,
            tc.tile_pool(name="pnat", bufs=2) as pnat_pool,
            tc.tile_pool(name="small", bufs=4) as sm_pool,
            tc.tile_pool(name="ostage", bufs=3) as ost_pool,
            tc.tile_pool(name="ps_qk", bufs=2, space="PSUM") as ps_qk,
            tc.tile_pool(name="ps_vw", bufs=1, space="PSUM") as ps_vw,
            tc.tile_pool(name="ps_s", bufs=2, space="PSUM") as ps_s,
            tc.tile_pool(name="ps_p9", bufs=1, space="PSUM") as ps_p9,
        ):
            cblob = cpool.tile([128, C_TOT], f32r, tag="cblob")
            nc.sync.dma_start(out=cblob[:], in_=c_blob[:])
            crow = cpool.tile([1, R_TOT], f32r, tag="crow")
            nc.sync.dma_start(out=crow[:], in_=c_row[:])
            ident_dr = cpool.tile([64, 2, 128], fp8e4, tag="idr")
            nc.sync.dma_start(out=ident_dr[:], in_=c_drl[:])
            mask_dr = cpool.tile([64, 2, 2, 256], fp8e4, tag="mdr")
            nc.sync.dma_start(out=mask_dr[:], in_=c_drm[:])

            msch = cblob[:, C_MSCH : C_MSCH + 512].bitcast(f32)
            maskf = cblob[:, C_ADJ : C_ADJ + 512]                 # [128, 512]
            identr = cblob[:, C_IDENT : C_IDENT + 128]
            mask2 = maskf.rearrange("p (c q) -> p c q", c=2)
            wqs = cblob[:, C_WQS : C_WQS + 128]                   # [128, 128]
            wks = cblob[:, C_WKS : C_WKS + 128]                   # [128, 128]
            wvo = cblob[:, C_WVO : C_WVO + 256]                   # [128, 256]
            bob = cblob[:, C_BO : C_BO + 8].bitcast(f32)          # [128, 8]
            bqv = cblob[:, C_BQ : C_BQ + 1].bitcast(f32)          # [128, 1]
            ones = crow[:, R_ONES : R_ONES + 64].bitcast(bf16)    # [1, 128]
            bvo = crow[:, R_BVO : R_BVO + 64].bitcast(bf16)       # [1, 128]

            # Make DVE/ACT/Pool observe the const-DMA queues once, so the
            # const-load ticks drop out of every later wait list.
            obs = cpool.tile([1, 8], f32, tag="obs")
            nc.vector.tensor_copy(obs[:, 0:2], cblob[0:1, 0:2].bitcast(f32))
            nc.vector.tensor_copy(obs[:, 2:4], crow[:, 0:2].bitcast(f32))
            nc.scalar.copy(obs[:, 4:6], cblob[0:1, 2:4].bitcast(f32))
            nc.scalar.copy(obs[:, 6:8], crow[:, 2:4].bitcast(f32))

            msch2 = msch.rearrange("p (c q) -> p c q", c=2)

            xt_tiles = {}

            def load_xt_group(g):
                t = xt_pool.tile([128, 4, 2, 128], f32r, tag="xt")
                nc.sync.dma_start(
                    out=t[:],
                    in_=x_t[4 * g : 4 * (g + 1)].rearrange(
                        "b f (c n) -> f b c n", c=2
                    ),
                )
                xt_tiles[g] = t

            def stage_compute(b):
                """Projections, copies, scores, exp for batch b (xt already
                loaded). P9(b-1) matmuls are emitted by the caller between
                proj and scores to keep PE dense."""
                xt_sb = xt_tiles[b // 4][:, b % 4]      # [128, 2, 128]
                xt_flat = xt_sb.rearrange("f c n -> f (c n)")

                ps_q = ps_qk.tile([128, 512], f32, tag="qk")
                nc.tensor.matmul(ps_q[:, 0:256], wqs, xt_flat,
                                 start=True, stop=False)
                i_projk = nc.tensor.matmul(ps_q[:, 256:512], wks, xt_flat,
                                           start=False, stop=True)
                # PE-order hint: run this batch's projections before the
                # previous batch's pair-0 P9 so the ACT qk-copy isn't starved
                _dep(p9_stage.get("p0_first"), i_projk, "proj before P9 p0")
                ps_v = ps_vw.tile([128, 512], f32, tag="vw")
                for c in range(2):
                    nc.tensor.matmul(
                        ps_v[:, 256 * c : 256 * (c + 1)], xt_sb[:, c, :], wvo,
                        start=(c == 0), stop=(c == 1),
                    )

                # qT/kT -> SBUF bf16 with the spread q bias applied per
                # partition (the +bq on kT rows adds a per-q-row constant to
                # scores, which softmax cancels).
                qk_sb = qk_pool.tile([128, 512], f32r, tag="qk")
                nc.scalar.activation(
                    qk_sb[:], ps_q[:, 0:512],
                    mybir.ActivationFunctionType.Identity, bias=bqv,
                )
                return {"qk": qk_sb, "ps_v": ps_v}

            def stage_vwcopy(st):
                vw_sb = vw_pool.tile([128, 2, 128], bf16, tag="vw")
                i_vw = nc.vector.tensor_copy(
                    vw_sb[:],
                    st["ps_v"][:].rearrange("p (c v) -> p c v", c=2)[:, :, 0:128],
                )
                _dep(i_vw, st.get("stt_last"), "vwcopy behind STT on DVE")
                # softmax row-sum ones columns (col 32h of each head group)
                nc.vector.memset(
                    vw_sb.rearrange("p c (h r) -> p c h r", r=32)[:, :, :, 0:1],
                    1.0,
                )
                st["vw"] = vw_sb

            def stage_scores(st, border):
                """Scores + exp for both pairs. The pair -> PSUM-slot
                assignment alternates per batch so the slot-release cycle
                interleaves the ACT-exp and DVE-STT drains (emission order
                stays fixed)."""
                e_parts = {}
                tiles = {}
                for p in border:
                    t = ps_s.tile([128, 2, 2, 256], f32, tag="S")
                    tiles[p] = t
                for p in range(2):
                    qk_sb = st["qk"]
                    ps_sp = tiles[p]
                    for hh in range(2):
                        h = 2 * p + hh
                        for c in range(2):
                            nc.tensor.matmul(
                                ps_sp[:, hh, c, :],
                                qk_sb[32 * h : 32 * h + 8,
                                      256 + 128 * c : 384 + 128 * c],
                                qk_sb[32 * h : 32 * h + 8, 0:256],
                                start=(c == 0),
                                stop=(p == 1 and c == 1),
                                skip_group_check=(p == 0 and c == 1),
                                tile_position=(32 * h, 0),
                            )
                    if p == 0:
                        # additive MASK_NEG plane accumulated after the
                        # scores (identity matmul); closes each bank's group
                        for hh in range(2):
                            for c in range(2):
                                nc.tensor.matmul(
                                    ps_sp[:, hh, c, :],
                                    ident_dr[:], mask_dr[:, c],
                                    perf_mode=mybir.MatmulPerfMode.DoubleRow,
                                    start=False, stop=(c == 1),
                                    skip_group_check=(c == 0),
                                )
                        e_raw = e_pool.tile([128, 2, 2, 256], bf16, tag="E0")
                        i_exp = nc.scalar.activation(
                            e_raw[:], ps_sp[:], mybir.ActivationFunctionType.Exp
                        )
                        _dep(i_exp, p9_stage.get("copy_last"),
                             "exp after previous p9copy in the ACT queue")
                        e_parts[0] = e_raw
                    else:
                        # DVE Schraudolph bf16-exp with mask folded into msch
                        e_i = e_pool.tile([128, 2, 2, 256], i16, tag="E1")
                        i_stt = nc.vector.scalar_tensor_tensor(
                            e_i[:], ps_sp[:], float(A16),
                            msch2.unsqueeze(1).to_broadcast([128, 2, 2, 256]),
                            mybir.AluOpType.mult, mybir.AluOpType.add,
                        )
                        st["stt_last"] = i_stt
                        e_parts[1] = e_i.bitcast(bf16)
                st["e"] = e_parts

            p9_stage = {}

            def stage_p9_mm(st, ps_mix, p):
                e_p = st["e"][p]
                first = None
                for hh in range(2):
                    h = 2 * p + hh
                    for c in range(2):
                        i_mm = nc.tensor.matmul(
                            ps_mix[32 * h : 32 * h + 32, 0:256],
                            st["vw"][:, c, 32 * h : 32 * h + 32],
                            e_p[:, hh, c, :],
                            start=(c == 0), stop=(c == 1),
                            tile_position=(0, 32 * h),
                        )
                        first = first or i_mm
                if p == 0:
                    p9_stage["p0_first"] = first

            def stage_p9_fin(j):
                """bf16 copy into the 4-batch staging tile; one DMA xbar
                transpose per 4 batches."""
                ps_mix = p9_stage["mix"]
                if j % 4 == 0:
                    p9s = p9_pool.tile([128, 4, 256], bf16, tag="p9")
                    p9_stage["t"] = p9s
                i_cp = nc.scalar.copy(p9_stage["t"][:, j % 4, :], ps_mix[:, 0:256])
                p9_stage["copy_last"] = i_cp
                if j % 4 == 3:
                    pn4 = pnat_pool.tile([128, 8, 128], bf16, tag="pn")
                    # issue from the ACT hwdge queue: same in-order queue as
                    # the staging copies, so the read can't race them
                    nc.scalar.dma_start_transpose(
                        out=pn4[:],
                        in_=p9_stage["t"].rearrange("p b q -> p (b q)"),
                    )
                    return pn4
                return None

            ostage = {}

            def stage_norm(pn4, g, after=None):
                """Normalize + head-sum + bias for batch group g (4 batches);
                DMA out every 2 groups."""
                pn_r = pn4.rearrange("p bc (h r) -> p bc h r", r=32)
                rec = sm_pool.tile([128, 8, H], f32, tag="rec")
                i_rec = nc.vector.reciprocal(rec[:], pn_r[:, :, :, 0])
                _dep(i_rec, after, "keep norm behind this batch's STT on DVE")
                tmp = sm_pool.tile([128, 8, D, H], f32, tag="tmp")
                nc.vector.tensor_mul(
                    tmp[:],
                    pn_r[:, :, :, 1:9].transpose([0, 1, 3, 2]),
                    rec[:].unsqueeze(2).to_broadcast([128, 8, D, H]),
                )
                red = sm_pool.tile([128, 8, D], f32, tag="red")
                nc.vector.tensor_reduce(
                    red[:], tmp[:], axis=mybir.AxisListType.X,
                    op=mybir.AluOpType.add,
                )
                if g % 2 == 0:
                    ost = ost_pool.tile([128, 8, 2, D], f32, tag="ost")
                    ostage["t"] = ost
                o = 4 * (g % 2)
                nc.vector.tensor_add(
                    ostage["t"][:, o : o + 4, :, :],
                    red[:].rearrange("p (b c) d -> p b c d", c=2),
                    bob.unsqueeze(1).unsqueeze(1).to_broadcast([128, 4, 2, D]),
                )
                if g % 2 == 1:
                    j0 = 8 * (g // 2)
                    nc.sync.dma_start(
                        out=out[j0 : j0 + 8].rearrange(
                            "b (c p) j -> p b c j", c=2
                        ),
                        in_=ostage["t"][:],
                    )

            # software pipeline: iteration b emits
            #   xt prefetch | proj(b) | P9(b-1) | scores(b) | norm(group)
            load_xt_group(0)
            prev_st = None
            pend_pn = []   # [(pn4, group)] awaiting normalize
            for b in range(BPC):
                if (b + 2) % 4 == 0 and (b + 2) // 4 < BPC // 4:
                    load_xt_group((b + 2) // 4)
                st = stage_compute(b)
                if prev_st is not None:
                    mix = ps_p9.tile([128, 512], f32, tag="p9")
                    p9_stage["mix"] = mix
                    stage_p9_mm(prev_st, mix, 1)
                stage_scores(st, (0, 1) if b % 2 == 0 else (1, 0))
                stage_vwcopy(st)
                if prev_st is not None:
                    stage_p9_mm(prev_st, p9_stage["mix"], 0)
                    pn4 = stage_p9_fin(b - 1)
                    if pn4 is not None:
                        pend_pn.append((pn4, (b - 1) // 4))
                # normalize with slack behind the transpose to cover DMA
                # latency
                if len(pend_pn) >= 2 or (pend_pn and b % 4 == 1):
                    pn4, g = pend_pn.pop(0)
                    stage_norm(pn4, g, st.get("stt_last"))
                prev_st = st
            mix = ps_p9.tile([128, 512], f32, tag="p9")
            p9_stage["mix"] = mix
            stage_p9_mm(prev_st, mix, 1)
            stage_p9_mm(prev_st, mix, 0)
            pn4 = stage_p9_fin(BPC - 1)
            if pn4 is not None:
                pend_pn.append((pn4, (BPC - 1) // 4))
            for pn4, g in pend_pn:
                stage_norm(pn4, g)

    _split_excess_waits(nc)
    return nc


_NC_CACHE = None
LAST_RESULTS = None


def kernel(**inputs) -> np.ndarray:
    global _NC_CACHE
    x = np.asarray(inputs["x"], np.float32)
    edge_index = np.asarray(inputs["edge_index"])
    consts = _build_consts(
        edge_index,
        np.asarray(inputs["Wq"], np.float32), np.asarray(inputs["bq"], np.float32),
        np.asarray(inputs["Wk"], np.float32), np.asarray(inputs["bk"], np.float32),
        np.asarray(inputs["Wv"], np.float32), np.asarray(inputs["bv"], np.float32),
        np.asarray(inputs["Wo"], np.float32), np.asarray(inputs["bo"], np.float32),
    )

    if _NC_CACHE is None:
        _NC_CACHE = _build_program()
    nc = _NC_CACHE

    in_maps = []
    for core in range(NCORES):
        xs = x[core * BPC : (core + 1) * BPC]  # [BPC, N, F]
        xt = np.ascontiguousarray(xs.transpose(0, 2, 1)).astype(np.float32)
        m = {"xt": xt}
        m.update(consts)
        m["cdrl"] = consts["cdrl"]
        m["cdrm"] = consts["cdrm"]
        in_maps.append(m)

    res = run_bass_kernel_spmd(nc, in_maps, list(range(NCORES)))
    global LAST_RESULTS
    LAST_RESULTS = res
    outs = [res.results[i]["out"] for i in range(NCORES)]
    return np.concatenate(outs, axis=0).astype(np.float32)


if __name__ == "__main__":
    rng = np.random.default_rng(0)
    demo = dict(
        x=rng.standard_normal((B, N, F), dtype=np.float32),
        edge_index=np.concatenate(
            [rng.integers(0, B, (2, 8192)), np.stack([np.arange(B)] * 2)], axis=1
        ).astype(np.int32),
        Wq=rng.standard_normal((F, H * D), dtype=np.float32) / np.sqrt(F),
        bq=rng.standard_normal(H * D, dtype=np.float32) / np.sqrt(F),
        Wk=rng.standard_normal((F, H * D), dtype=np.float32) / np.sqrt(F),
        bk=rng.standard_normal(H * D, dtype=np.float32) / np.sqrt(F),
        Wv=rng.standard_normal((F, H * D), dtype=np.float32) / np.sqrt(F),
        bv=rng.standard_normal(H * D, dtype=np.float32) / np.sqrt(F),
        Wo=rng.standard_normal((H * D, D), dtype=np.float32) / np.sqrt(H * D),
        bo=rng.standard_normal(D, dtype=np.float32) / np.sqrt(H * D),
    )
    out = kernel(**demo)
    print("kernel output", out.shape, out.dtype)


# revision 55
# speedup vs baseline: 1.1065x; 1.0201x over previous
"""GAT-style dense attention kernel for TRN2 (8 NeuronCores, SPMD over batch).

Reference computation (B=N=256, F=128, H=4, D=8):
  q = x@Wq+bq; k = x@Wk+bk; v = x@Wv+bv          (per-head dim D=8)
  s = einsum('bqhd,bkhd->bhqk', q, k)/sqrt(D)
  s = where(adj[q,k]==0, -inf, s)                 (adj shared across b,h)
  a = softmax(s, -1)
  out = einsum('bhqk,bkhd->bqhd', a, v).reshape(B,N,H*D) @ Wo + bo

Kernel strategy (per core: 32 batches):
  - host: xT = x.transpose -> [b, F, N] in bf16 so contraction dim F is on
    partitions; all projection weights bf16 (spread layout: head h occupies
    partitions 32h..32h+8; scale 1/sqrt(D) folded into Wq/bq)
  - bias algebra: softmax over k is invariant to per-q-row constants, so the
    k-projection bias drops entirely and the q bias rides the PSUM->SBUF
    copy as a per-partition ACT bias operand (the spurious +bq on the kT
    partitions adds a per-q constant to scores - also softmax-invariant);
    bv@Wo is constant post-normalization and folds into bo
  - scores S^T[k,q] per head: K=8 matmuls, 4 heads in PE row groups
  - exp split across engines, mask folded in per-path:
      pair 0 (heads 0,1): additive -20 mask preloaded in PSUM via one
        identity-matmul per head, then ACT Exp -> bf16
      pair 1 (heads 2,3): DVE scalar_tensor_tensor Schraudolph exp:
        i16 = trunc(S*A16 + msch) bitcast bf16, with the mask and the
        truncation bias folded into the msch plane (no PE mask matmuls)
  - V and Wo fused on host: Wvo_h = Wv_h @ Wo_h; a ones column per head in
    the same stationary operand yields the softmax row-sums
  - P9 matmuls col-packed: head h writes PSUM partitions 32h..32h+9
  - P9 -> bf16 (ACT copy), then DMA xbar transpose back to natural [q, :]
    layout; VectorE: reciprocal of rowsums, scale, sum heads, +bo, DMA out
"""

import sys

sys.path.insert(0, "/opt/trn_rl_repo")

import ml_dtypes
import numpy as np

import concourse.bass as bass
import concourse.tile as tile
from concourse import mybir
from concourse.bass_utils import run_bass_kernel_spmd
from concourse.tile_rust import add_dep_helper


def _dep(from_inst, to_inst, reason):
    if from_inst is None or to_inst is None:
        return
    add_dep_helper(
        getattr(from_inst, "ins", from_inst),
        getattr(to_inst, "ins", to_inst),
        sync=False,
        reason=reason,
    )

B = 256
N = 256
F = 128
H = 4
D = 8
NCORES = 8
BPC = B // NCORES  # batches per core
MASK_NEG = -20.0
# Schraudolph bf16-exp constants: i16 = trunc(s * A16 + B16) bitcast bf16.
# B16 includes +0.5 so that truncation acts as round-to-nearest.
A16 = 184.6618
B16 = 16250.5

f32 = mybir.dt.float32
f32r = mybir.dt.float32r
bf16 = mybir.dt.bfloat16
i16 = mybir.dt.int16

# cblob f32-column layout
C_MSCH = 0           # [128, 512] f32 schraudolph plane (c, q), pair-1 path
C_ADJ = 512          # [128, 512] f32r additive mask (c, q) MASK_NEG/0
C_IDENT = 1024       # [128, 128] f32r identity (mask matmul stationary)
C_WQS = 1152         # [128, 128] f32r
C_WKS = 1280         # [128, 128] f32r
C_WVO = 1408         # [128, 256] f32r (cols 128:256 zero pad)
C_BO = 1664          # [128, 8] f32 (bo + sum_h bv_h@Wo_h, bcast to partitions)
C_BQ = 1672          # [128, 1] f32 spread q bias (ACT bias operand)
C_IDR = 1673         # [64p, 2, 128] fp8e4 interleaved identity -> 64 f32 cols
C_MDR = 1737         # [64p, 2, 2, 256] fp8e4 interleaved mask -> 256 f32 cols
C_TOT = 1993
# crow f32-column layout
R_ONES = 0           # [1, 128] bf16 -> 64 f32 cols
R_BVO = 64           # [1, 128] bf16 -> 64 f32 cols (ones columns only)
R_TOT = 128


def _pack_fp8(a):
    """Pack a [r, 4c] array into [r, c] f32 via float8_e4m3fn bytes."""
    b = np.ascontiguousarray(a.astype(ml_dtypes.float8_e4m3fn)).view(np.uint8)
    packed = (b[:, 0::4].astype(np.uint32)
              | (b[:, 1::4].astype(np.uint32) << 8)
              | (b[:, 2::4].astype(np.uint32) << 16)
              | (b[:, 3::4].astype(np.uint32) << 24))
    return packed.view(np.float32)


def _pack_bf16(a):
    """Pack a [r, 2c] bf16 array into [r, c] f32 (low half = even cols)."""
    b = np.ascontiguousarray(a.astype(ml_dtypes.bfloat16)).view(np.uint16)
    packed = b[:, 0::2].astype(np.uint32) | (b[:, 1::2].astype(np.uint32) << 16)
    return packed.view(np.float32)


def _build_consts(edge_index, Wq, bq, Wk, bk, Wv, bv, Wo, bo):
    scale = 1.0 / np.sqrt(np.float32(D))

    # spread projection weights: output partition 32h+d holds head h, dim d
    Wq_s = np.zeros((F, 128), np.float32)
    Wk_s = np.zeros((F, 128), np.float32)
    bq_s = np.zeros((128, 1), np.float32)
    for h in range(H):
        for d in range(D):
            Wq_s[:, 32 * h + d] = Wq[:, 8 * h + d] * scale
            Wk_s[:, 32 * h + d] = Wk[:, 8 * h + d]
            bq_s[32 * h + d, 0] = bq[8 * h + d] * scale

    # fused V*Wo, head h occupies cols 32h..32h+31: col 32h gets the ones
    # (from the crow ones-matmul), cols 32h+1..8 the fused V@Wo, the rest
    # zero padding (so P9 writes whole 32-partition groups). bv@Wo folds
    # into the output bias.
    Wvo = np.zeros((F, 128), np.float32)
    bo_eff = bo.astype(np.float32).copy()
    for h in range(H):
        wv_h = Wv[:, 8 * h : 8 * h + 8]  # [F, 8]
        wo_h = Wo[8 * h : 8 * h + 8, :]  # [8, 8]
        Wvo[:, 32 * h + 1 : 32 * h + 9] = wv_h @ wo_h
        bo_eff += bv[8 * h : 8 * h + 8] @ wo_h
    bvo_ones = np.zeros((1, 128), np.float32)
    for h in range(H):
        bvo_ones[0, 32 * h] = 1.0

    # adjacency; mask addend M^T[k, q] packed as [128, (c, q)]
    adj = np.zeros((B, B), np.float32)
    adj[edge_index[0], edge_index[1]] = 1.0
    adjT_p = np.ascontiguousarray(
        adj.T.reshape(2, 128, 256).transpose(1, 0, 2)
    ).reshape(128, 512)
    maskT_p = np.where(adjT_p != 0.0, np.float32(0.0), np.float32(MASK_NEG))
    msch = np.where(
        adj.T == 0.0, np.float32(B16 + A16 * MASK_NEG), np.float32(B16)
    )
    msch_p = np.ascontiguousarray(
        msch.reshape(2, 128, 256).transpose(1, 0, 2)
    ).reshape(128, 512)

    bo_b = np.broadcast_to(bo_eff, (128, D)).copy()

    cblob = np.zeros((128, C_TOT), np.float32)
    cblob[:, C_MSCH : C_MSCH + 512] = msch_p
    cblob[:, C_ADJ : C_ADJ + 512] = maskT_p
    cblob[:, C_IDENT : C_IDENT + 128] = np.eye(128, dtype=np.float32)
    # fp8e4 DoubleRow operands: partition k holds rows 2k/2k+1 interleaved
    cblob[0:64, C_IDR : C_IDR + 64] = _pack_fp8(
        np.eye(128, dtype=np.float32).reshape(64, 256)
    )
    mask_dr = np.stack(
        [maskT_p[:, 256 * c : 256 * (c + 1)].reshape(64, 512) for c in range(2)],
        axis=1,
    ).reshape(64, 1024)
    cblob[0:64, C_MDR : C_MDR + 256] = _pack_fp8(mask_dr)
    cblob[:, C_WQS : C_WQS + 128] = Wq_s
    cblob[:, C_WKS : C_WKS + 128] = Wk_s
    cblob[:, C_WVO : C_WVO + 128] = Wvo
    cblob[:, C_BO : C_BO + 8] = bo_b
    cblob[:, C_BQ : C_BQ + 1] = bq_s

    ident_drn = np.eye(128, dtype=np.float32).reshape(64, 2, 128)
    mask_drn = np.stack(
        [maskT_p[:, 256 * c : 256 * (c + 1)].reshape(64, 2, 256) for c in range(2)],
        axis=1,
    )  # [64, 2(c), 2(i), 256]

    crow = np.zeros((1, R_TOT), np.float32)
    crow[:, R_ONES : R_ONES + 64] = _pack_bf16(np.ones((1, 128), np.float32))
    crow[:, R_BVO : R_BVO + 64] = _pack_bf16(bvo_ones)
    return dict(
        cblob=np.ascontiguousarray(cblob), crow=np.ascontiguousarray(crow),
        cdrl=np.ascontiguousarray(ident_drn).astype(ml_dtypes.float8_e4m3fn),
        cdrm=np.ascontiguousarray(mask_drn).astype(ml_dtypes.float8_e4m3fn),
    )


def _split_excess_waits(nc, max_waits=1):
    """Walrus allows only 2 sync-wait slots per engine instruction. Tile's
    vector-clock wait emission occasionally exceeds that (schedule-dependent);
    hoist the excess onto injected same-engine NoOps placed just before."""
    f = nc.m.functions[0]
    for bb in f.blocks:
        insts = list(bb.instructions)
        n_inserted = 0
        for idx, inst in enumerate(insts):
            si = getattr(inst, "sync_info", None)
            if si is None or not si.on_wait or len(si.on_wait) <= max_waits:
                continue
            waits = list(si.on_wait)
            keep, excess = waits[:max_waits], waits[max_waits:]
            pos = idx + n_inserted
            while excess:
                chunk, excess = excess[:max_waits], excess[max_waits:]
                nop = mybir.InstNoOp(
                    name=nc.get_next_instruction_name(),
                    ins=[],
                    outs=[],
                    engine=inst.engine,
                    sync_info=mybir.SyncInfo(on_wait=chunk, on_update=[]),
                    bass_nofuse=True,
                )
                bb.instructions.insert(pos, nop)
                pos += 1
                n_inserted += 1
            inst.sync_info = mybir.SyncInfo(on_wait=keep, on_update=si.on_update)


def _build_program():
    nc = bass.Bass()

    x_t = nc.declare_dram_parameter("xt", [BPC, F, N], f32r, isOutput=False)
    out = nc.declare_dram_parameter("out", [BPC, N, D], f32, isOutput=True)
    c_blob = nc.declare_dram_parameter("cblob", [128, C_TOT], f32r, isOutput=False)
    c_row = nc.declare_dram_parameter("crow", [1, R_TOT], f32r, isOutput=False)
    fp8e4 = mybir.dt.float8e4
    c_drl = nc.declare_dram_parameter("cdrl", [64, 2, 128], fp8e4, isOutput=False)
    c_drm = nc.declare_dram_parameter("cdrm", [64, 2, 2, 256], fp8e4, isOutput=False)

    with tile.TileContext(nc) as tc:
        with (
            tc.tile_pool(name="consts", bufs=1) as cpool,
            tc.tile_pool(name="xt", bufs=3) as xt_pool,
            tc.tile_pool(name="qk", bufs=3) as qk_pool,
            tc.tile_pool(name="vw", bufs=3) as vw_pool,
            tc.tile_pool(name="E", bufs=3) as e_pool,
            tc.tile_pool(name="p9", bufs=2) as p9_pool,
            tc.tile_pool(name="pnat", bufs=2) as pnat_pool,
            tc.tile_pool(name="small", bufs=4) as sm_pool,
            tc.tile_pool(name="ostage", bufs=3) as ost_pool,
            tc.tile_pool(name="ps_qk", bufs=2, space="PSUM") as ps_qk,
            tc.tile_pool(name="ps_vw", bufs=1, space="PSUM") as ps_vw,
            tc.tile_pool(name="ps_s", bufs=2, space="PSUM") as ps_s,
            tc.tile_pool(name="ps_p9", bufs=1, space="PSUM") as ps_p9,
        ):
            xt0 = xt_pool.tile([128, 4, 2, 128], f32r, tag="xt")
            nc.sync.dma_start(
                out=xt0[:],
                in_=x_t[0:4].rearrange("b f (c n) -> f b c n", c=2),
            )
            cblob = cpool.tile([128, C_TOT], f32r, tag="cblob")
            nc.sync.dma_start(
                out=cblob[:, C_WQS:C_TOT], in_=c_blob[:, C_WQS:C_TOT]
            )
            nc.sync.dma_start(
                out=cblob[:, C_MSCH : C_MSCH + 512],
                in_=c_blob[:, C_MSCH : C_MSCH + 512],
            )
            crow = cpool.tile([1, R_TOT], f32r, tag="crow")
            nc.sync.dma_start(out=crow[:], in_=c_row[:])
            ident_dr = cpool.tile([64, 2, 128], fp8e4, tag="idr")
            nc.sync.dma_start(out=ident_dr[:], in_=c_drl[:])
            mask_dr = cpool.tile([64, 2, 2, 256], fp8e4, tag="mdr")
            nc.sync.dma_start(out=mask_dr[:], in_=c_drm[:])

            msch = cblob[:, C_MSCH : C_MSCH + 512].bitcast(f32)
            maskf = cblob[:, C_ADJ : C_ADJ + 512]                 # [128, 512]
            identr = cblob[:, C_IDENT : C_IDENT + 128]
            mask2 = maskf.rearrange("p (c q) -> p c q", c=2)
            wqs = cblob[:, C_WQS : C_WQS + 128]                   # [128, 128]
            wks = cblob[:, C_WKS : C_WKS + 128]                   # [128, 128]
            wvo = cblob[:, C_WVO : C_WVO + 256]                   # [128, 256]
            bob = cblob[:, C_BO : C_BO + 8].bitcast(f32)          # [128, 8]
            bqv = cblob[:, C_BQ : C_BQ + 1].bitcast(f32)          # [128, 1]
            ones = crow[:, R_ONES : R_ONES + 64].bitcast(bf16)    # [1, 128]
            bvo = crow[:, R_BVO : R_BVO + 64].bitcast(bf16)       # [1, 128]

            # Make DVE/ACT/Pool observe the const-DMA queues once, so the
            # const-load ticks drop out of every later wait list.
            obs = cpool.tile([1, 12], f32, tag="obs")
            nc.vector.tensor_copy(obs[:, 0:2], cblob[0:1, 0:2].bitcast(f32))
            nc.vector.tensor_copy(
                obs[:, 2:4], cblob[0:1, C_WQS : C_WQS + 2].bitcast(f32)
            )
            nc.vector.tensor_copy(obs[:, 4:6], crow[:, 0:2].bitcast(f32))
            nc.scalar.copy(obs[:, 6:8], cblob[0:1, 2:4].bitcast(f32))
            nc.scalar.copy(
                obs[:, 8:10], cblob[0:1, C_WQS + 2 : C_WQS + 4].bitcast(f32)
            )
            nc.scalar.copy(obs[:, 10:12], crow[:, 2:4].bitcast(f32))

            msch2 = msch.rearrange("p (c q) -> p c q", c=2)

            xt_tiles = {0: xt0}

            def load_xt_group(g):
                if g in xt_tiles:
                    return
                t = xt_pool.tile([128, 4, 2, 128], f32r, tag="xt")
                nc.sync.dma_start(
                    out=t[:],
                    in_=x_t[4 * g : 4 * (g + 1)].rearrange(
                        "b f (c n) -> f b c n", c=2
                    ),
                )
                xt_tiles[g] = t

            def stage_compute(b):
                """Projections, copies, scores, exp for batch b (xt already
                loaded). P9(b-1) matmuls are emitted by the caller between
                proj and scores to keep PE dense."""
                xt_sb = xt_tiles[b // 4][:, b % 4]      # [128, 2, 128]
                xt_flat = xt_sb.rearrange("f c n -> f (c n)")

                ps_q = ps_qk.tile([128, 512], f32, tag="qk")
                nc.tensor.matmul(ps_q[:, 0:256], wqs, xt_flat,
                                 start=True, stop=False)
                i_projk = nc.tensor.matmul(ps_q[:, 256:512], wks, xt_flat,
                                           start=False, stop=True)
                # PE-order hint: run this batch's projections before the
                # previous batch's pair-0 P9 so the ACT qk-copy isn't starved
                _dep(p9_stage.get("p0_first"), i_projk, "proj before P9 p0")
                ps_v = ps_vw.tile([128, 512], f32, tag="vw")
                for c in range(2):
                    nc.tensor.matmul(
                        ps_v[:, 256 * c : 256 * (c + 1)], xt_sb[:, c, :], wvo,
                        start=(c == 0), stop=(c == 1),
                    )

                # qT/kT -> SBUF bf16 with the spread q bias applied per
                # partition (the +bq on kT rows adds a per-q-row constant to
                # scores, which softmax cancels).
                qk_sb = qk_pool.tile([128, 512], f32r, tag="qk")
                nc.scalar.activation(
                    qk_sb[:], ps_q[:, 0:512],
                    mybir.ActivationFunctionType.Identity, bias=bqv,
                )
                return {"qk": qk_sb, "ps_v": ps_v}

            def stage_vwcopy(st):
                vw_sb = vw_pool.tile([128, 2, 128], bf16, tag="vw")
                i_vw = nc.vector.tensor_copy(
                    vw_sb[:],
                    st["ps_v"][:].rearrange("p (c v) -> p c v", c=2)[:, :, 0:128],
                )
                _dep(i_vw, st.get("stt_last"), "vwcopy behind STT on DVE")
                # softmax row-sum ones columns (col 32h of each head group)
                nc.vector.memset(
                    vw_sb.rearrange("p c (h r) -> p c h r", r=32)[:, :, :, 0:1],
                    1.0,
                )
                st["vw"] = vw_sb

            def stage_scores(st, border):
                """Scores + exp for both pairs. The pair -> PSUM-slot
                assignment alternates per batch so the slot-release cycle
                interleaves the ACT-exp and DVE-STT drains (emission order
                stays fixed)."""
                e_parts = {}
                tiles = {}
                for p in border:
                    t = ps_s.tile([128, 2, 2, 256], f32, tag="S")
                    tiles[p] = t
                for p in range(2):
                    qk_sb = st["qk"]
                    ps_sp = tiles[p]
                    for hh in range(2):
                        h = 2 * p + hh
                        for c in range(2):
                            nc.tensor.matmul(
                                ps_sp[:, hh, c, :],
                                qk_sb[32 * h : 32 * h + 8,
                                      256 + 128 * c : 384 + 128 * c],
                                qk_sb[32 * h : 32 * h + 8, 0:256],
                                start=(c == 0),
                                stop=(p == 1 and c == 1),
                                skip_group_check=(p == 0 and c == 1),
                                tile_position=(32 * h, 0),
                            )
                    if p == 0:
                        # additive MASK_NEG plane accumulated after the
                        # scores (identity matmul); closes each bank's group
                        for hh in range(2):
                            for c in range(2):
                                nc.tensor.matmul(
                                    ps_sp[:, hh, c, :],
                                    ident_dr[:], mask_dr[:, c],
                                    perf_mode=mybir.MatmulPerfMode.DoubleRow,
                                    start=False, stop=(c == 1),
                                    skip_group_check=(c == 0),
                                )
                        e_raw = e_pool.tile([128, 2, 2, 256], bf16, tag="E0")
                        i_exp = nc.scalar.activation(
                            e_raw[:], ps_sp[:], mybir.ActivationFunctionType.Exp
                        )
                        _dep(i_exp, p9_stage.get("copy_last"),
                             "exp after previous p9copy in the ACT queue")
                        e_parts[0] = e_raw
                    else:
                        # DVE Schraudolph bf16-exp with mask folded into msch
                        e_i = e_pool.tile([128, 2, 2, 256], i16, tag="E1")
                        i_stt = nc.vector.scalar_tensor_tensor(
                            e_i[:], ps_sp[:], float(A16),
                            msch2.unsqueeze(1).to_broadcast([128, 2, 2, 256]),
                            mybir.AluOpType.mult, mybir.AluOpType.add,
                        )
                        st["stt_last"] = i_stt
                        e_parts[1] = e_i.bitcast(bf16)
                st["e"] = e_parts

            p9_stage = {}

            def stage_p9_mm(st, ps_mix, p):
                e_p = st["e"][p]
                first = None
                for hh in range(2):
                    h = 2 * p + hh
                    for c in range(2):
                        i_mm = nc.tensor.matmul(
                            ps_mix[32 * h : 32 * h + 32, 0:256],
                            st["vw"][:, c, 32 * h : 32 * h + 32],
                            e_p[:, hh, c, :],
                            start=(c == 0), stop=(c == 1),
                            tile_position=(0, 32 * h),
                        )
                        first = first or i_mm
                if p == 0:
                    p9_stage["p0_first"] = first

            def stage_p9_fin(j):
                """bf16 copy into the 4-batch staging tile; one DMA xbar
                transpose per 4 batches."""
                ps_mix = p9_stage["mix"]
                if j % 4 == 0:
                    p9s = p9_pool.tile([128, 4, 256], bf16, tag="p9")
                    p9_stage["t"] = p9s
                i_cp = nc.scalar.copy(p9_stage["t"][:, j % 4, :], ps_mix[:, 0:256])
                p9_stage["copy_last"] = i_cp
                if j % 4 == 3:
                    pn4 = pnat_pool.tile([128, 8, 128], bf16, tag="pn")
                    # issue from the ACT hwdge queue: same in-order queue as
                    # the staging copies, so the read can't race them
                    nc.scalar.dma_start_transpose(
                        out=pn4[:],
                        in_=p9_stage["t"].rearrange("p b q -> p (b q)"),
                    )
                    return pn4
                return None

            ostage = {}

            def stage_norm(pn4, g, after=None):
                """Normalize + head-sum + bias for batch group g (4 batches);
                DMA out every 2 groups."""
                pn_r = pn4.rearrange("p bc (h r) -> p bc h r", r=32)
                rec = sm_pool.tile([128, 8, H], f32, tag="rec")
                i_rec = nc.vector.reciprocal(rec[:], pn_r[:, :, :, 0])
                _dep(i_rec, after, "keep norm behind this batch's STT on DVE")
                tmp = sm_pool.tile([128, 8, D, H], f32, tag="tmp")
                nc.vector.tensor_mul(
                    tmp[:],
                    pn_r[:, :, :, 1:9].transpose([0, 1, 3, 2]),
                    rec[:].unsqueeze(2).to_broadcast([128, 8, D, H]),
                )
                red = sm_pool.tile([128, 8, D], f32, tag="red")
                nc.vector.tensor_reduce(
                    red[:], tmp[:], axis=mybir.AxisListType.X,
                    op=mybir.AluOpType.add,
                )
                ost = ost_pool.tile([128, 4, 2, D], f32, tag="ost")
                nc.vector.tensor_add(
                    ost[:],
                    red[:].rearrange("p (b c) d -> p b c d", c=2),
                    bob.unsqueeze(1).unsqueeze(1).to_broadcast([128, 4, 2, D]),
                )
                nc.sync.dma_start(
                    out=out[4 * g : 4 * g + 4].rearrange(
                        "b (c p) j -> p b c j", c=2
                    ),
                    in_=ost[:],
                )

            # software pipeline: iteration b emits
            #   xt prefetch | proj(b) | P9(b-1) | scores(b) | norm(group)
            load_xt_group(0)
            prev_st = None
            pend_pn = []   # [(pn4, group)] awaiting normalize
            for b in range(BPC):
                if (b + 2) % 4 == 0 and (b + 2) // 4 < BPC // 4:
                    load_xt_group((b + 2) // 4)
                st = stage_compute(b)
                if prev_st is not None:
                    mix = ps_p9.tile([128, 512], f32, tag="p9")
                    p9_stage["mix"] = mix
                    stage_p9_mm(prev_st, mix, 1)
                stage_scores(st, (0, 1) if b % 2 == 0 else (1, 0))
                stage_vwcopy(st)
                if prev_st is not None:
                    stage_p9_mm(prev_st, p9_stage["mix"], 0)
                    pn4 = stage_p9_fin(b - 1)
                    if pn4 is not None:
                        pend_pn.append((pn4, (b - 1) // 4))
                # normalize with slack behind the transpose to cover DMA
                # latency
                if len(pend_pn) >= 2 or (pend_pn and b % 4 == 1):
                    pn4, g = pend_pn.pop(0)
                    stage_norm(pn4, g, st.get("stt_last"))
                prev_st = st
            mix = ps_p9.tile([128, 512], f32, tag="p9")
            p9_stage["mix"] = mix
            stage_p9_mm(prev_st, mix, 1)
            stage_p9_mm(prev_st, mix, 0)
            pn4 = stage_p9_fin(BPC - 1)
            if pn4 is not None:
                pend_pn.append((pn4, (BPC - 1) // 4))
            for pn4, g in pend_pn:
                stage_norm(pn4, g)

    _split_excess_waits(nc)
    return nc


_NC_CACHE = None
LAST_RESULTS = None


def kernel(**inputs) -> np.ndarray:
    global _NC_CACHE
    x = np.asarray(inputs["x"], np.float32)
    edge_index = np.asarray(inputs["edge_index"])
    consts = _build_consts(
        edge_index,
        np.asarray(inputs["Wq"], np.float32), np.asarray(inputs["bq"], np.float32),
        np.asarray(inputs["Wk"], np.float32), np.asarray(inputs["bk"], np.float32),
        np.asarray(inputs["Wv"], np.float32), np.asarray(inputs["bv"], np.float32),
        np.asarray(inputs["Wo"], np.float32), np.asarray(inputs["bo"], np.float32),
    )

    if _NC_CACHE is None:
        _NC_CACHE = _build_program()
    nc = _NC_CACHE

    in_maps = []
    for core in range(NCORES):
        xs = x[core * BPC : (core + 1) * BPC]  # [BPC, N, F]
        xt = np.ascontiguousarray(xs.transpose(0, 2, 1)).astype(np.float32)
        m = {"xt": xt}
        m.update(consts)
        m["cdrl"] = consts["cdrl"]
        m["cdrm"] = consts["cdrm"]
        in_maps.append(m)

    res = run_bass_kernel_spmd(nc, in_maps, list(range(NCORES)))
    global LAST_RESULTS
    LAST_RESULTS = res
    outs = [res.results[i]["out"] for i in range(NCORES)]
    return np.concatenate(outs, axis=0).astype(np.float32)


if __name__ == "__main__":
    rng = np.random.default_rng(0)
    demo = dict(
        x=rng.standard_normal((B, N, F), dtype=np.float32),
        edge_index=np.concatenate(
            [rng.integers(0, B, (2, 8192)), np.stack([np.arange(B)] * 2)], axis=1
        ).astype(np.int32),
        Wq=rng.standard_normal((F, H * D), dtype=np.float32) / np.sqrt(F),
        bq=rng.standard_normal(H * D, dtype=np.float32) / np.sqrt(F),
        Wk=rng.standard_normal((F, H * D), dtype=np.float32) / np.sqrt(F),
        bk=rng.standard_normal(H * D, dtype=np.float32) / np.sqrt(F),
        Wv=rng.standard_normal((F, H * D), dtype=np.float32) / np.sqrt(F),
        bv=rng.standard_normal(H * D, dtype=np.float32) / np.sqrt(F),
        Wo=rng.standard_normal((H * D, D), dtype=np.float32) / np.sqrt(H * D),
        bo=rng.standard_normal(D, dtype=np.float32) / np.sqrt(H * D),
    )
    out = kernel(**demo)
    print("kernel output", out.shape, out.dtype)
